# revision 1
# baseline (speedup 1.0000x reference)
"""Trainium2 Bass kernel for nn_LocalKConv (KAN conv block).

Pipeline per batch image (one batch per NeuronCore, 8 cores):
  LN1 -> tanh basis (T0=1, T1=t, T2=2t^2-1) -> 3x3 conv (384ch) -> 1x1 conv
  -> +bias -> +input -> LN2 -> gelu -> +input

Device strategy:
  * 1x1 conv folded into the KAN conv weights on host (exact linear algebra).
  * T0 (all-ones) basis group folded into a 9-region bias table applied via a
    tiny K=9 indicator matmul (exact: conv of the in-image ones mask).
  * T2 = 2t^2-1 rewritten as basis t^2 with weights x2 and the "-1" folded
    into the bias table (exact).
  * Conv weights output-centered on host so the conv PSUM directly holds
    h - mean_ch(h) (xc = x - mean(x) is channel-centered, bias table also
    centered) -> LN2 needs only one stats matmul (variance).
  * rstd = 1/(sqrt(v)+eps) via ACT splines: rho=exp(-0.5 ln v), then
    r = rho - eps*rho^2 (error O(eps^2) ~ 1e-10).
  * Conv operands in bf16 (fp32 matmul is two-pass on TRN2), fp32 PSUM
    accumulate; stats/bias matmuls stay fp32.
"""

import sys

if "/opt/trn_rl_repo" not in sys.path:
    sys.path.insert(0, "/opt/trn_rl_repo")

import numpy as np
from contextlib import ExitStack

B, C, H, W = 8, 128, 56, 56
HW = H * W            # 3136
PH = H + 2            # 58 padded
NCORES = 8
CHROWS = 7            # output rows per matmul chunk
NCHUNK = H // CHROWS  # 8
CHPX = CHROWS * W     # 392 pixels per chunk
BLKCH = 2             # chunks per elementwise block
NBLK = NCHUNK // BLKCH  # 4
BLKPX = BLKCH * CHPX  # 784
EPS = 1e-5

_cached = {}


def _host_prep(kan_w, conv2_w, conv2_b, ln_g, ln_b):
    """Fold 1x1 conv, build centered bf16 weights, bias9 table, indicator."""
    C2 = conv2_w.reshape(C, C).astype(np.float64)
    Wf = np.einsum("oc,cikl->oikl", C2, kan_w.astype(np.float64))  # [co,384,3,3]
    W0 = Wf[:, 0:C]          # ones group
    W1 = Wf[:, C:2 * C]      # t group
    W2 = Wf[:, 2 * C:3 * C]  # (2t^2-1) group
    W2s = 2.0 * W2           # t^2 basis gets 2x weight

    # ones-plane kernel: +1*W0 (T0) and -1*W2 (from 2t^2-1) on in-image ones
    S = (W0 - W2).sum(axis=1)  # [co, 3, 3]
    # region types: 0=first row/col, 1=interior, 2=last; valid dy sets
    vsets = {0: (1, 2), 1: (0, 1, 2), 2: (0, 1)}
    bias9 = np.zeros((9, C), np.float64)
    for ty in range(3):
        for tx in range(3):
            acc = np.zeros(C, np.float64)
            for dy in vsets[ty]:
                for dx in vsets[tx]:
                    acc += S[:, dy, dx]
            bias9[ty * 3 + tx] = acc + conv2_b.astype(np.float64)

    # output-center (over co) so conv PSUM holds h - mean_ch(h)
    W1c = W1 - W1.mean(axis=0, keepdims=True)
    W2c = W2s - W2s.mean(axis=0, keepdims=True)
    b9c = bias9 - bias9.mean(axis=1, keepdims=True)

    # lhsT layout [ci, slot*co]; slot s = g*9 + dy*3 + dx
    wt = np.empty((C, 18 * C), np.float32)
    for g, Wg in enumerate((W1c, W2c)):
        for t in range(9):
            dy, dx = t // 3, t % 3
            s = g * 9 + t
            wt[:, s * C:(s + 1) * C] = Wg[:, :, dy, dx].T.astype(np.float32)

    yy = np.arange(H)
    ty = np.where(yy == 0, 0, np.where(yy == H - 1, 2, 1))
    tx = np.where(yy == 0, 0, np.where(yy == W - 1, 2, 1))
    reg = (ty[:, None] * 3 + tx[None, :]).reshape(-1)  # [3136]
    ind = np.zeros((9, HW), np.float32)
    ind[reg, np.arange(HW)] = 1.0

    lnp = np.stack([ln_g.reshape(C), ln_b.reshape(C)], axis=1).astype(np.float32)
    return {
        "w": wt.astype(np.dtype("bfloat16") if False else np.float32),  # cast below
        "wt_bf16": wt,
        "b9": b9c.astype(np.float32),
        "ind": ind,
        "lnp": lnp,
    }


def _build_program():
    import concourse.bacc as bacc
    import concourse.mybir as mybir
    import concourse.tile as tile
    from concourse.tile import add_dep_helper

    AF = mybir.ActivationFunctionType
    OP = mybir.AluOpType
    F32 = mybir.dt.float32
    BF16 = mybir.dt.bfloat16

    nc = bacc.Bacc("TRN2", target_bir_lowering=False, debug=False)

    # extra float consts for activation scale immediates
    for val in (-0.5,):
        t = nc.alloc_sbuf_tensor(f"constx-f32-{val}", [128, 1], F32)
        nc.gpsimd.memset(t.ap(), val)
        nc.const_aps.aps[(F32, val)] = t.ap()

    x_d = nc.dram_tensor("x", [C, HW], F32, kind="ExternalInput")
    w_d = nc.dram_tensor("w", [C, 18 * C], BF16, kind="ExternalInput")
    b9_d = nc.dram_tensor("b9", [9, C], F32, kind="ExternalInput")
    ind_d = nc.dram_tensor("ind", [9, HW], F32, kind="ExternalInput")
    lnp_d = nc.dram_tensor("lnp", [C, 2], F32, kind="ExternalInput")
    y_d = nc.dram_tensor("y", [C, HW], F32, kind="ExternalOutput")

    with tile.TileContext(nc) as tc, ExitStack() as ctx:
        cpool = ctx.enter_context(tc.tile_pool(name="const", bufs=1))
        ipool = ctx.enter_context(tc.tile_pool(name="img", bufs=1))
        bpool = ctx.enter_context(tc.tile_pool(name="blk", bufs=2))
        epool = ctx.enter_context(tc.tile_pool(name="epi", bufs=2))
        pstat = ctx.enter_context(tc.tile_pool(name="pstat", bufs=2, space="PSUM"))
        pconv = ctx.enter_context(tc.tile_pool(name="pconv", bufs=4, space="PSUM"))

        w_sb = cpool.tile([C, 18 * C], BF16)
        nc.sync.dma_start(w_sb[:], w_d.ap())
        b9_sb = cpool.tile([9, C], F32)
        nc.sync.dma_start(b9_sb[:], b9_d.ap())
        ind_sb = cpool.tile([9, HW], F32)
        nc.sync.dma_start(ind_sb[:], ind_d.ap())
        lnp_sb = cpool.tile([C, 2], F32)
        nc.sync.dma_start(lnp_sb[:], lnp_d.ap())
        ones_sb = cpool.tile([C, C], F32)
        nc.vector.memset(ones_sb[:], 1.0 / C)
        ones_bf = cpool.tile([C, C], BF16)
        nc.vector.memset(ones_bf[:], 1.0 / C)

        x_sb = ipool.tile([C, HW], F32)
        xc_sb = ipool.tile([C, HW], F32)
        tpad = ipool.tile([C, PH * PH], BF16)
        t2pad = ipool.tile([C, PH * PH], BF16)
        tpv = tpad.rearrange("p (a b) -> p a b", a=PH)
        t2pv = t2pad.rearrange("p (a b) -> p a b", a=PH)
        # zero borders (top/bottom rows, left/right cols)
        for v in (tpv, t2pv):
            nc.vector.memset(v[:, 0, :], 0.0)
            nc.vector.memset(v[:, PH - 1, :], 0.0)
            nc.vector.memset(v[:, 1:PH - 1, 0], 0.0)
            nc.vector.memset(v[:, 1:PH - 1, PH - 1], 0.0)

        indv = ind_sb.rearrange("k (h w) -> k h w", h=H)
        g_ap = lnp_sb[:, 0:1]
        b_ap = lnp_sb[:, 1:2]

        Y1, X1, Y2, X2 = [], [], [], []
        hs_tiles = {}

        # block DMAs of x
        for b in range(NBLK):
            px = slice(b * BLKPX, (b + 1) * BLKPX)
            nc.sync.dma_start(x_sb[:, px], x_d.ap()[:, px])

        # ---------------- interleaved wavefront emission ----------------
        def emit_ln1(b):
            px = slice(b * BLKPX, (b + 1) * BLKPX)
            xbf = bpool.tile([C, BLKPX], BF16, name=f"xbf{b}", tag="xbf")
            nc.vector.tensor_copy(xbf[:], x_sb[:, px])
            Pm = pstat.tile([C, 2, 512], F32, name=f"Pm{b}", tag="stat")
            for j in range(2):
                nc.tensor.matmul(Pm[:, j, 0:CHPX], ones_bf[:],
                                 xbf[:, j * CHPX:(j + 1) * CHPX],
                                 start=True, stop=True)
            xcv = xc_sb[:, px].rearrange("p (a b) -> p a b", a=2)
            xv = x_sb[:, px].rearrange("p (a b) -> p a b", a=2)
            nc.vector.tensor_tensor(xcv, xv, Pm[:, :, 0:CHPX], OP.subtract)
            xcsq = bpool.tile([C, BLKPX], BF16, name=f"xcsq{b}", tag="xcsq")
            nc.vector.tensor_tensor(xcsq[:], xc_sb[:, px], xc_sb[:, px], OP.mult)
            Pv = pstat.tile([C, 2, 512], F32, name=f"Pv{b}", tag="stat")
            for j in range(2):
                nc.tensor.matmul(Pv[:, j, 0:CHPX], ones_bf[:],
                                 xcsq[:, j * CHPX:(j + 1) * CHPX],
                                 start=True, stop=True)
            a_t = bpool.tile([C, BLKPX], F32, name=f"a{b}", tag="a")
            av = a_t.rearrange("p (a b) -> p a b", a=2)
            Y1.append(nc.scalar.activation(av, Pv[:, :, 0:CHPX], AF.Ln))
            rho = bpool.tile([C, BLKPX], F32, name=f"rho{b}", tag="rho")
            Y1.append(nc.scalar.activation(rho[:], a_t[:], AF.Exp, scale=-0.5))
            xn = bpool.tile([C, BLKPX], F32, name=f"xn{b}", tag="xn")
            nc.vector.tensor_tensor(xn[:], xc_sb[:, px], rho[:], OP.mult)
            rows = slice(14 * b + 1, 14 * b + 15)
            xnv = xn.rearrange("p (a b) -> p a b", a=14)
            X1.append(nc.scalar.activation(tpv[:, rows, 1:W + 1], xnv, AF.Tanh,
                                           bias=b_ap, scale=g_ap))
            nc.vector.tensor_tensor(t2pv[:, rows, 1:W + 1], tpv[:, rows, 1:W + 1],
                                    tpv[:, rows, 1:W + 1], OP.mult)

        def emit_conv(chunks):
            Pc = {}
            for c in chunks:
                Pc[c] = pconv.tile([C, CHPX], F32, name=f"Pc{c}", tag="conv")
                pv = Pc[c].rearrange("p (a b) -> p a b", a=CHROWS)
                nc.tensor.matmul(pv, b9_sb[:], indv[:, CHROWS * c:CHROWS * (c + 1), :],
                                 start=True, stop=False)
            for s in range(18):
                g, t = s // 9, s % 9
                dy, dx = t // 3, t % 3
                src = tpv if g == 0 else t2pv
                for c in chunks:
                    pv = Pc[c].rearrange("p (a b) -> p a b", a=CHROWS)
                    rhs = src[:, CHROWS * c + dy:CHROWS * c + dy + CHROWS, dx:dx + W]
                    nc.tensor.matmul(pv, w_sb[:, s * C:(s + 1) * C], rhs,
                                     start=False, stop=(s == 17))
            for c in chunks:
                hs_tiles[c] = Pc[c]

        def emit_epi(b):
            px = slice(b * BLKPX, (b + 1) * BLKPX)
            hs = epool.tile([C, BLKPX], F32, name=f"hs{b}", tag="hs")
            for j in range(2):
                c = 2 * b + j
                cs = slice(c * CHPX, (c + 1) * CHPX)
                nc.vector.tensor_tensor(hs[:, j * CHPX:(j + 1) * CHPX],
                                        hs_tiles[c][:], xc_sb[:, cs], OP.add)
            hsq = epool.tile([C, BLKPX], BF16, name=f"hsq{b}", tag="hsq")
            nc.vector.tensor_tensor(hsq[:], hs[:], hs[:], OP.mult)
            Pv2 = pstat.tile([C, 2, 512], F32, name=f"Pv2{b}", tag="stat")
            for j in range(2):
                nc.tensor.matmul(Pv2[:, j, 0:CHPX], ones_bf[:],
                                 hsq[:, j * CHPX:(j + 1) * CHPX],
                                 start=True, stop=True)
            a2 = epool.tile([C, BLKPX], F32, name=f"a2{b}", tag="a2")
            a2v = a2.rearrange("p (a b) -> p a b", a=2)
            Y2.append(nc.scalar.activation(a2v, Pv2[:, :, 0:CHPX], AF.Ln))
            rho2 = epool.tile([C, BLKPX], F32, name=f"rho2{b}", tag="rho2")
            Y2.append(nc.scalar.activation(rho2[:], a2[:], AF.Exp, scale=-0.5))
            xn2 = epool.tile([C, BLKPX], F32, name=f"xn2{b}", tag="xn2")
            nc.vector.tensor_tensor(xn2[:], hs[:], rho2[:], OP.mult)
            ge = epool.tile([C, BLKPX], F32, name=f"ge{b}", tag="ge")
            X2.append(nc.scalar.activation(ge[:], xn2[:], AF.Gelu,
                                           bias=b_ap, scale=g_ap))
            outt = epool.tile([C, BLKPX], F32, name=f"out{b}", tag="out")
            nc.gpsimd.tensor_tensor(outt[:], ge[:], x_sb[:, px], OP.add)
            nc.sync.dma_start(y_d.ap()[:, px], outt[:])

        # wavefront: conv chunk c ready once basis rows <= 7c+8 are written
        emit_ln1(0)
        emit_conv([0])
        emit_ln1(1)
        emit_conv([1, 2])
        emit_ln1(2)
        emit_epi(0)
        emit_conv([3, 4])
        emit_ln1(3)
        emit_epi(1)
        emit_conv([5, 6, 7])
        emit_epi(2)
        emit_epi(3)

        # ACT table-set batching within block pairs
        def pair_edges(Ys, Xs):
            # Ys/Xs indexed per block; batch pairs (0,1) and (2,3)
            for p in (0, 2):
                ys = Ys[2 * p:2 * p + 4] if len(Ys) == 8 else Ys[p:p + 2]
                xs = Xs[p:p + 2]
                for xi in xs:
                    for yi in ys:
                        add_dep_helper(xi.ins, yi.ins, sync=False)
        pair_edges(Y1, X1)
        pair_edges(Y2, X2)

    nc.compile()
    return nc


def kernel(input_tensor, ln_g, ln_b, kan_w, conv2_w, conv2_b):
    from concourse.bass_utils import run_bass_kernel_spmd
    import ml_dtypes

    prep = _host_prep(np.asarray(kan_w), np.asarray(conv2_w),
                      np.asarray(conv2_b), np.asarray(ln_g), np.asarray(ln_b))
    if "nc" not in _cached:
        _cached["nc"] = _build_program()
    nc = _cached["nc"]

    w_bf = prep["wt_bf16"].astype(ml_dtypes.bfloat16)
    x = np.asarray(input_tensor)
    in_maps = []
    for b in range(NCORES):
        in_maps.append({
            "x": np.ascontiguousarray(x[b].reshape(C, HW), dtype=np.float32),
            "w": w_bf,
            "b9": prep["b9"],
            "ind": prep["ind"],
            "lnp": prep["lnp"],
        })
    res = run_bass_kernel_spmd(nc, in_maps, list(range(NCORES)),
                               trace=_cached.get("trace", False))
    _cached["exec_time_ns"] = res.exec_time_ns
    out = np.stack([res.results[b]["y"].reshape(C, H, W) for b in range(NCORES)])
    return out.astype(np.float32)

